# revision 40
# baseline (speedup 1.0000x reference)
"""Distance-attention kernel for Trainium2, SPMD over 8 NeuronCores.

Reference computation (per batch element b):
    q = x @ Wq.T ; k = x @ Wk.T ; v = x @ Wv.T          [S, F]
    scores = cdist(q, k) / sqrt(768)                     [S, S]
    attn = softmax(scores)   (softmax of RAW distances)
    out = attn @ v                                       [S, F]

Sharding: data-parallel over batch. B == 8 == n_cores, so core b computes
batch element b end-to-end; weights are replicated. No collectives.

Device algorithm (matmul inputs bf16, fp32 PSUM accumulation):

  d2[j, i] - 1536 = (k2[j]-768) + (q2[i]-768) - 2*qk[j, i] comes from ONE
  augmented GEMM: K-chunks 0..5 hold (-2*k)^T against q^T; two K=1 chunks
  add the norm rows: (k2-768)[j] x ones[i] and ones[j] x (q2-768)[i].
  All four aug operands live on partition 0 of their tensors (engines
  cannot address partition >0 as an op start, and DMA instructions only
  support a single semaphore wait in this toolchain, so no partition-1
  staging DMAs are possible).  Centering by 768 (~E[q2]) keeps the
  bf16-stored aug rows accurate.

  scores = sqrt(d2/768) is needed only inside a softmax, so any additive
  constant cancels.  Over the realizable d2 range ([~1060, ~2160]; fit on
  [900, 2300]) a minimax quadratic  s ~ a*(w+h)^2 + C  (w = d2-1536,
  a < 0) is accurate to 4e-3, below bf16 logit noise.  Dropping C:
      p = exp(s - C) = exp(-(alpha*w + beta)^2)
  -> one fused DVE tensor_scalar (m = alpha*w + beta), one DVE square,
  one ACT Exp(scale=-1).  No Sqrt on ACT at all, so the ACT LUT never
  switches table sets inside the hot loop (a Sqrt/Exp alternation would
  cost ~5.3us per iteration in table loads).

  attn@v and the softmax denominator come from ONE GEMM against v
  augmented with a ones column: out_psum[i, 0:768] = sum_j p*v,
  out_psum[i, 768] = sum_j p.  Epilogue: out = out_psum[:, :768] *
  (1/out_psum[:, 768]), written bf16 (host upcasts to f32).

DMA discipline (walrus rejects DMA instructions with >1 semaphore wait):
inputs arrive pre-cast to bf16 (no on-device cast producer), every load
DMA targets a fresh never-reused tile in a fresh SBUF zone (zero waits),
and stores read a persistent staging buffer placed in the zone vacated by
the weight-transpose pool (compute-only history => the store's only wait
is its DVE producer).
"""

import numpy as np
import ml_dtypes
from contextlib import ExitStack

import concourse.bass as bass
import concourse.tile as tile
from concourse import mybir
from concourse.bass_utils import run_bass_kernel_spmd
from concourse.masks import make_identity

S, D, F = 2048, 768, 768
P = 128
SC = S // P          # 16 token chunks
DC = D // P          # 6 contraction chunks for QKV
FC = F // P          # 6
IG = 256             # i-group width in the main loop
NIG = S // IG        # 8
NI = IG // P         # 2 i-chunks per group
FV = F + 1           # v columns + ones column
C2 = 768.0           # centering constant for q2/k2 rows
BF16 = mybir.dt.bfloat16
F32 = mybir.dt.float32
AF = mybir.ActivationFunctionType
ALU = mybir.AluOpType

# minimax quadratic for sqrt((w+1536)/768), w = d2-1536 in [900,2300]-1536:
# s ~ -(ALPHA*w + BETA)^2 + C  =>  softmax weights p = exp(-(ALPHA*w+BETA)^2)
ALPHA = 1.929106785463655e-04
BETA = -1.2116557543348982

N_CORES = 8

XCH = 4              # x loaded in 4 chunks of 4 token-blocks
WCH = 3              # each W loaded in 3 chunks of 2 row-blocks


def build_bass(target_bir_lowering: bool = False) -> bass.Bass:
    nc = bass.Bass(target_bir_lowering=target_bir_lowering)
    x_ext = nc.declare_dram_parameter("x", [S, D], BF16, isOutput=False)
    wq_ext = nc.declare_dram_parameter("Wq", [F, D], BF16, isOutput=False)
    wk_ext = nc.declare_dram_parameter("Wk", [F, D], BF16, isOutput=False)
    wv_ext = nc.declare_dram_parameter("Wv", [F, D], BF16, isOutput=False)
    out_ext = nc.declare_dram_parameter("out", [S, F], BF16, isOutput=True)

    with tile.TileContext(nc) as tc, ExitStack() as ctx:
        big = ctx.enter_context(tc.tile_pool(name="big", bufs=1))
        xT = big.tile([P, DC, S], BF16)       # x^T   [d, i] (chunked d)
        qTa = big.tile([P, DC, S], BF16)      # q^T          [f, i]
        kTa = big.tile([P, DC, S], BF16)      # -2*k^T       [f, j]
        vA = big.tile([P, SC, FV], BF16)      # v (+ ones col)  [j, f+1]
        # sum weights for the norm reduction matmul: col 0 -> 1.0 (q2 from
        # q^2), col 1 -> 0.25 (k2 from (-2k)^2)
        sumw = big.tile([P, 2], BF16)
        q2row = big.tile([1, S], BF16)        # (q2-768)[i] on partition 0
        k2row = big.tile([1, S], BF16)        # (k2-768)[j] on partition 0
        ones_row = big.tile([1, S], BF16)     # ones[j] on partition 0
        k2term = big.tile([P, SC], F32)       # ALPHA*(k2-768)[j] + BETA
        ident = big.tile([P, P], BF16)
        make_identity(nc, ident[:])

        nc.vector.memset(sumw[:, 0:1], 1.0)
        nc.vector.memset(sumw[:, 1:2], 0.25)
        nc.vector.memset(ones_row[:], 1.0)
        nc.vector.memset(vA[:, :, F : F + 1], 1.0)

        # Short PE warm-up burst so the HAM clock is at 2.4 GHz when the
        # first transposes issue.
        wdum = big.tile([P, 512], BF16)
        nc.vector.memset(wdum[:], 1.0)
        with tc.tile_pool(name="warm_ps", bufs=1, space="PSUM") as warm_ps:
            wps = warm_ps.tile([P, 512], F32)
            for _ in range(16):
                nc.tensor.matmul(
                    wps[:], wdum[:, 0:P], wdum[:], start=True, stop=True
                )

        # weight transposes live only through phase 2; phase 3 reuses their
        # SBUF zone for the output staging buffer.
        wt_pool = tc.tile_pool(name="wt", bufs=1)
        with wt_pool as wt:
            wqT = wt.tile([P, DC, F], BF16)
            wkT = wt.tile([P, DC, F], BF16)
            wvT = wt.tile([P, DC, F], BF16)

            # ---------- phase 1: load (bf16 from host), PE-transpose ------
            with tc.tile_pool(name="stage", bufs=1) as stage, \
                 tc.tile_pool(name="tr_ps", bufs=4, space="PSUM") as tr_ps:

                def transpose_block(src, dst, t):
                    # src: [P, D] natural bf16 -> dst[:, c, t*P:(t+1)*P]
                    for c in range(DC):
                        tp = tr_ps.tile([P, P], BF16, name="tp")
                        nc.tensor.transpose(
                            tp[:], src[:, c * P : (c + 1) * P], ident[:]
                        )
                        cp = (
                            nc.scalar.copy
                            if c % 2 == 0
                            else nc.vector.tensor_copy
                        )
                        cp(out=dst[:, c, t * P : (t + 1) * P], in_=tp[:])

                def load_w(ext, dst, wname):
                    nblk = FC // WCH  # 2 row-blocks per chunk
                    for ch in range(WCH):
                        wld = stage.tile(
                            [P, nblk, D], BF16, name=f"{wname}{ch}"
                        )
                        nc.gpsimd.dma_start(
                            out=wld[:],
                            in_=ext.ap().rearrange("(c p) d -> p c d", p=P)[
                                :, ch * nblk : (ch + 1) * nblk, :
                            ],
                        )
                        for b in range(nblk):
                            transpose_block(wld[:, b, :], dst, ch * nblk + b)

                # Wq then x first: the q-projection GEMMs (emitted first in
                # phase 2) only need these, so PE goes dense ASAP.
                load_w(wq_ext, wqT, "wq")
                nblk = SC // XCH  # 4 token-blocks per chunk
                for ch in range(XCH):
                    xld = stage.tile([P, nblk, D], BF16, name=f"x{ch}")
                    nc.gpsimd.dma_start(
                        out=xld[:],
                        in_=x_ext.ap().rearrange("(t p) d -> p t d", p=P)[
                            :, ch * nblk : (ch + 1) * nblk, :
                        ],
                    )
                    for b in range(nblk):
                        transpose_block(xld[:, b, :], xT, ch * nblk + b)
                load_w(wk_ext, wkT, "wk")
                load_w(wv_ext, wvT, "wv")

            # ---------- phase 2: QKV projections (+ q2 / k2) ----------
            with tc.tile_pool(name="qkv_ps", bufs=4, space="PSUM") as qkv_ps, \
                 tc.tile_pool(name="n2_ps", bufs=2, space="PSUM") as n2_ps, \
                 tc.tile_pool(name="sq_sb", bufs=4) as sq_sb:
                # q and k projections: psum tile [128(f), 512(i)]; squares
                # reduce over f via a [128,1]-weights matmul into [1, 512].
                for which in ("q", "k"):
                    wT = wqT if which == "q" else wkT
                    dstT = qTa if which == "q" else kTa
                    wcol = slice(0, 1) if which == "q" else slice(1, 2)
                    for i4 in range(S // 512):
                        isl = slice(i4 * 512, (i4 + 1) * 512)
                        n2t = n2_ps.tile([1, 512], F32)
                        for fc in range(FC):
                            pp = qkv_ps.tile([P, 512], F32, name="proj")
                            for kc in range(DC):
                                nc.tensor.matmul(
                                    pp[:],
                                    wT[:, kc, fc * P : (fc + 1) * P],
                                    xT[:, kc, isl],
                                    start=(kc == 0),
                                    stop=(kc == DC - 1),
                                )
                            if which == "q":
                                nc.vector.tensor_copy(
                                    out=dstT[:, fc, isl], in_=pp[:]
                                )
                            else:
                                nc.scalar.mul(dstT[:, fc, isl], pp[:], -2.0)
                            sq = sq_sb.tile([P, 512], BF16)
                            nc.vector.tensor_mul(
                                sq[:], dstT[:, fc, isl], dstT[:, fc, isl]
                            )
                            nc.tensor.matmul(
                                n2t[:],
                                sumw[:, wcol],
                                sq[:],
                                start=(fc == 0),
                                stop=(fc == FC - 1),
                            )
                        dstrow = q2row if which == "q" else k2row
                        nc.vector.tensor_scalar_add(
                            dstrow[0:1, isl], n2t[0:1, :], -C2
                        )

                # v projection: psum tile [128(j), n(f)]
                for jt in range(SC):
                    for n0, nw in ((0, 512), (512, F - 512)):
                        vp = qkv_ps.tile([P, 512], F32, name="proj")
                        for kc in range(DC):
                            nc.tensor.matmul(
                                vp[:, 0:nw],
                                xT[:, kc, jt * P : (jt + 1) * P],
                                wvT[:, kc, n0 : n0 + nw],
                                start=(kc == 0),
                                stop=(kc == DC - 1),
                            )
                        nc.scalar.copy(
                            out=vA[:, jt, n0 : n0 + nw], in_=vp[:, 0:nw]
                        )

                # (k2-768)[j] partition-major via 16 tiny PE transposes
                # ([1,128] -> [128,1] psum columns), then one affine:
                # k2term[p, t] = ALPHA*(k2-768)[t*128+p] + BETA
                # column stride padded to 4B (PSUM writes must be aligned)
                k2ps = n2_ps.tile([P, SC, 2], BF16, name="k2ps")
                for t in range(SC):
                    nc.tensor.transpose(
                        k2ps[:, t, 0:1],
                        k2row[0:1, t * P : (t + 1) * P],
                        ident[0:1, 0:1],
                    )
                nc.vector.tensor_scalar(
                    k2term[:], k2ps[:, :, 0], ALPHA, BETA,
                    op0=ALU.mult, op1=ALU.add,
                )
        # wt pool closed: its zone (compute-written only) hosts o_buf now.

        # ---------- phase 3: distances -> softmax -> attn @ v ----------
        # jt pairs share one [128, 512] psum tile (halves hold two j-blocks
        # over the same 256 i-columns) so DVE/ACT run at 512-wide.
        with tc.tile_pool(name="o_pool", bufs=1) as o_pool, \
             tc.tile_pool(name="out_ps", bufs=1, space="PSUM") as out_ps_pool, \
             tc.tile_pool(name="d2_ps", bufs=3, space="PSUM") as d2_ps, \
             tc.tile_pool(name="m_sb", bufs=3) as m_sb, \
             tc.tile_pool(name="s2_sb", bufs=3) as s2_sb, \
             tc.tile_pool(name="p_sb", bufs=3) as p_sb, \
             tc.tile_pool(name="r_sb", bufs=3) as r_sb:
            o_buf = o_pool.tile([P, SC, F], BF16)
            for ig in range(NIG):
                igsl = slice(ig * IG, (ig + 1) * IG)
                outp = [
                    out_ps_pool.tile([P, FV], F32, name=f"outp{ic}")
                    for ic in range(NI)
                ]
                for jp in range(SC // 2):
                    d2 = d2_ps.tile([P, 2 * IG], F32)
                    for half in range(2):
                        jt = 2 * jp + half
                        jsl = slice(jt * P, (jt + 1) * P)
                        dsl = slice(half * IG, (half + 1) * IG)
                        for kc in range(DC):
                            nc.tensor.matmul(
                                d2[:, dsl],
                                kTa[:, kc, jsl],
                                qTa[:, kc, igsl],
                                start=(kc == 0),
                                stop=False,
                            )
                        # q2 row:  ones[j] x (q2-768)[i]  (k2 is folded into
                        # the per-partition affine below)
                        nc.tensor.matmul(
                            d2[:, dsl],
                            ones_row[0:1, jsl],
                            q2row[0:1, igsl],
                            start=False,
                            stop=True,
                        )
                    # m = ALPHA*(w + q2') + (ALPHA*k2'[j] + BETA); the two
                    # jt halves carry different per-partition k2 scalars.
                    m = m_sb.tile([P, 2 * IG], F32)
                    for half in range(2):
                        jt = 2 * jp + half
                        dsl = slice(half * IG, (half + 1) * IG)
                        nc.vector.tensor_scalar(
                            m[:, dsl],
                            d2[:, dsl],
                            ALPHA,
                            k2term[:, jt : jt + 1],
                            op0=ALU.mult,
                            op1=ALU.add,
                        )
                    s2 = s2_sb.tile([P, 2 * IG], F32)
                    nc.vector.tensor_mul(s2[:], m[:], m[:])
                    p = p_sb.tile([P, 2 * IG], BF16)
                    nc.scalar.activation(
                        out=p[:], in_=s2[:], func=AF.Exp, scale=-1.0
                    )
                    for half in range(2):
                        jt = 2 * jp + half
                        for ic in range(NI):
                            lhs = p[
                                :, half * IG + ic * P : half * IG + (ic + 1) * P
                            ]
                            for n0, nw in ((0, 512), (512, FV - 512)):
                                nc.tensor.matmul(
                                    outp[ic][:, n0 : n0 + nw],
                                    lhs,
                                    vA[:, jt, n0 : n0 + nw],
                                    start=(jt == 0),
                                    stop=(jt == SC - 1),
                                )
                for ic in range(NI):
                    tidx = ig * NI + ic
                    rcp = r_sb.tile([P, 1], F32)
                    nc.vector.reciprocal(rcp[:], outp[ic][:, F : F + 1])
                    nc.vector.tensor_scalar_mul(
                        o_buf[:, tidx, :], outp[ic][:, 0:F], rcp[:]
                    )
                    i0 = tidx * P
                    nc.gpsimd.dma_start(
                        out=out_ext[i0 : i0 + P, :], in_=o_buf[:, tidx, :]
                    )

    return nc


_DMA_TYPES = (
    "InstDMACopy",
    "InstDmaTransposeAnt",
    "InstDMA",
    "InstDMAGatherAnt",
    "InstDMAScatterAddAnt",
)
_SKIP_TYPES = (
    "InstUnconditionalBranch",
    "InstCall",
    "InstISA",
    "InstCompareAndBranch",
    "InstIndirectBranch",
    "InstHalt",
)


def _split_dma_waits(nc: bass.Bass) -> bass.Bass:
    """walrus' setupSyncWait accepts a limited number of embedded semaphore
    waits per instruction (1 for DMA, 2 for regular TPB instructions);
    hoist any extra waits into NoOps on the issuing engine (engine FIFO
    order makes them complete before the instruction issues)."""
    ctr = 0
    for bb in nc.m.functions[0].blocks:
        new_insts = []
        for ins in bb.instructions:
            t = type(ins).__name__
            si = ins.sync_info
            cap = 1
            if t not in _SKIP_TYPES and si is not None and len(si.on_wait) > cap:
                for w in si.on_wait[:-cap]:
                    ctr += 1
                    new_insts.append(
                        mybir.InstNoOp(
                            name=f"antwait-{ctr}",
                            engine=ins.engine,
                            sync_info=mybir.SyncInfo(on_wait=[w], on_update=[]),
                        )
                    )
                ins.sync_info = mybir.SyncInfo(
                    on_wait=list(si.on_wait[-cap:]), on_update=list(si.on_update)
                )
            new_insts.append(ins)
        bb.instructions = new_insts
    return nc


_CACHED_NC = None


def _get_nc():
    global _CACHED_NC
    if _CACHED_NC is None:
        _CACHED_NC = _split_dma_waits(build_bass())
    return _CACHED_NC


def _bf16(a):
    return np.asarray(a, dtype=np.float32).astype(ml_dtypes.bfloat16)


def kernel(x, Wq, Wk, Wv, _trace=False):
    x = _bf16(x)
    Wq = np.ascontiguousarray(_bf16(Wq))
    Wk = np.ascontiguousarray(_bf16(Wk))
    Wv = np.ascontiguousarray(_bf16(Wv))
    assert x.shape == (N_CORES, S, D)

    nc = _get_nc()
    in_maps = [
        {"x": np.ascontiguousarray(x[b]), "Wq": Wq, "Wk": Wk, "Wv": Wv}
        for b in range(N_CORES)
    ]
    res = run_bass_kernel_spmd(
        nc, in_maps, core_ids=list(range(N_CORES)), trace=_trace
    )
    out = np.stack(
        [res.results[b]["out"].astype(np.float32) for b in range(N_CORES)],
        axis=0,
    )
    if _trace:
        return out, res
    return out


# revision 41
# speedup vs baseline: 1.1885x; 1.1885x over previous
"""Distance-attention kernel for Trainium2, SPMD over 8 NeuronCores.

Reference computation (per batch element b):
    q = x @ Wq.T ; k = x @ Wk.T ; v = x @ Wv.T          [S, F]
    scores = cdist(q, k) / sqrt(768)                     [S, S]
    attn = softmax(scores)   (softmax of RAW distances)
    out = attn @ v                                       [S, F]

Sharding: data-parallel over batch. B == 8 == n_cores, so core b computes
batch element b end-to-end; weights are replicated. No collectives.

Device algorithm (matmul inputs bf16, fp32 PSUM accumulation):

  d2[j, i] - 1536 = (k2[j]-768) + (q2[i]-768) - 2*qk[j, i] comes from ONE
  augmented GEMM: K-chunks 0..5 hold (-2*k)^T against q^T; two K=1 chunks
  add the norm rows: (k2-768)[j] x ones[i] and ones[j] x (q2-768)[i].
  All four aug operands live on partition 0 of their tensors (engines
  cannot address partition >0 as an op start, and DMA instructions only
  support a single semaphore wait in this toolchain, so no partition-1
  staging DMAs are possible).  Centering by 768 (~E[q2]) keeps the
  bf16-stored aug rows accurate.

  scores = sqrt(d2/768) is needed only inside a softmax, so any additive
  constant cancels.  Over the realizable d2 range ([~1060, ~2160]; fit on
  [900, 2300]) a minimax quadratic  s ~ a*(w+h)^2 + C  (w = d2-1536,
  a < 0) is accurate to 4e-3, below bf16 logit noise.  Dropping C:
      p = exp(s - C) = exp(-(alpha*w + beta)^2)
  -> one fused DVE tensor_scalar (m = alpha*w + beta), one DVE square,
  one ACT Exp(scale=-1).  No Sqrt on ACT at all, so the ACT LUT never
  switches table sets inside the hot loop (a Sqrt/Exp alternation would
  cost ~5.3us per iteration in table loads).

  attn@v and the softmax denominator come from ONE GEMM against v
  augmented with a ones column: out_psum[i, 0:768] = sum_j p*v,
  out_psum[i, 768] = sum_j p.  Epilogue: out = out_psum[:, :768] *
  (1/out_psum[:, 768]), written bf16 (host upcasts to f32).

DMA discipline (walrus rejects DMA instructions with >1 semaphore wait):
inputs arrive pre-cast to bf16 (no on-device cast producer), every load
DMA targets a fresh never-reused tile in a fresh SBUF zone (zero waits),
and stores read a persistent staging buffer placed in the zone vacated by
the weight-transpose pool (compute-only history => the store's only wait
is its DVE producer).
"""

import numpy as np
import ml_dtypes
from contextlib import ExitStack

import concourse.bass as bass
import concourse.tile as tile
from concourse import mybir
from concourse.bass_utils import run_bass_kernel_spmd
from concourse.masks import make_identity

S, D, F = 2048, 768, 768
P = 128
SC = S // P          # 16 token chunks
DC = D // P          # 6 contraction chunks for QKV
FC = F // P          # 6
IG = 256             # i-group width in the main loop
NIG = S // IG        # 8
NI = IG // P         # 2 i-chunks per group
FV = F + 1           # v columns + ones column
C2 = 768.0           # centering constant for q2/k2 rows
BF16 = mybir.dt.bfloat16
FP8 = mybir.dt.float8e4
F32 = mybir.dt.float32
AF = mybir.ActivationFunctionType
ALU = mybir.AluOpType

# minimax quadratic for sqrt((w+1536)/768), w = d2-1536 in [900,2300]-1536:
# s ~ -(ALPHA*w + BETA)^2 + C  =>  softmax weights p = exp(-(ALPHA*w+BETA)^2)
ALPHA = 1.929106785463655e-04
BETA = -1.2116557543348982

N_CORES = 8

XCH = 4              # x loaded in 4 chunks of 4 token-blocks
WCH = 3              # each W loaded in 3 chunks of 2 row-blocks


def build_bass(target_bir_lowering: bool = False) -> bass.Bass:
    nc = bass.Bass(target_bir_lowering=target_bir_lowering)
    x_ext = nc.declare_dram_parameter("x", [S, D], BF16, isOutput=False)
    wq_ext = nc.declare_dram_parameter("Wq", [F, D], BF16, isOutput=False)
    wk_ext = nc.declare_dram_parameter("Wk", [F, D], BF16, isOutput=False)
    wv_ext = nc.declare_dram_parameter("Wv", [F, D], BF16, isOutput=False)
    out_ext = nc.declare_dram_parameter("out", [S, F], BF16, isOutput=True)

    with tile.TileContext(nc) as tc, ExitStack() as ctx:
        big = ctx.enter_context(tc.tile_pool(name="big", bufs=1))
        xT = big.tile([P, DC, S], BF16)       # x^T   [d, i] (chunked d)
        qTa = big.tile([P, DC, S], FP8)       # q^T  (fp8 for DoubleRow)
        kTa = big.tile([P, DC, S], FP8)       # -2*k^T (fp8 for DoubleRow)
        vA = big.tile([P, SC, FV], BF16)      # v (+ ones col)  [j, f+1]
        # sum weights for the norm reduction matmul: col 0 -> 1.0 (q2 from
        # q^2), col 1 -> 0.25 (k2 from (-2k)^2)
        sumw = big.tile([P, 2], BF16)
        q2row = big.tile([1, S], BF16)        # (q2-768)[i] on partition 0
        k2row = big.tile([1, S], BF16)        # (k2-768)[j] on partition 0
        ones_row = big.tile([1, S], BF16)     # ones on partition 0
        ident = big.tile([P, P], BF16)
        make_identity(nc, ident[:])

        nc.vector.memset(sumw[:, 0:1], 1.0)
        nc.vector.memset(sumw[:, 1:2], 0.25)
        nc.vector.memset(ones_row[:], 1.0)
        nc.vector.memset(vA[:, :, F : F + 1], 1.0)

        # weight transposes live only through phase 2; phase 3 reuses their
        # SBUF zone for the output staging buffer.
        wt_pool = tc.tile_pool(name="wt", bufs=1)
        with wt_pool as wt:
            wqT = wt.tile([P, DC, F], BF16)
            wkT = wt.tile([P, DC, F], BF16)
            wvT = wt.tile([P, DC, F], BF16)

            # ---------- phase 1: load (bf16 from host), PE-transpose ------
            with tc.tile_pool(name="stage", bufs=1) as stage, \
                 tc.tile_pool(name="tr_ps", bufs=2, space="PSUM") as tr_ps:

                def transpose_block(src, dst, t):
                    # src: [P, D] natural bf16 -> dst[:, c, t*P:(t+1)*P]
                    for c in range(DC):
                        tp = tr_ps.tile([P, P], BF16, name="tp")
                        nc.tensor.transpose(
                            tp[:], src[:, c * P : (c + 1) * P], ident[:]
                        )
                        cp = (
                            nc.scalar.copy
                            if c % 2 == 0
                            else nc.vector.tensor_copy
                        )
                        cp(out=dst[:, c, t * P : (t + 1) * P], in_=tp[:])

                for ext, dst, wname in (
                    (wq_ext, wqT, "wq"),
                    (wk_ext, wkT, "wk"),
                    (wv_ext, wvT, "wv"),
                ):
                    nblk = FC // WCH  # 2 row-blocks per chunk
                    for ch in range(WCH):
                        wld = stage.tile(
                            [P, nblk, D], BF16, name=f"{wname}{ch}"
                        )
                        nc.gpsimd.dma_start(
                            out=wld[:],
                            in_=ext.ap().rearrange("(c p) d -> p c d", p=P)[
                                :, ch * nblk : (ch + 1) * nblk, :
                            ],
                        )
                        for b in range(nblk):
                            transpose_block(wld[:, b, :], dst, ch * nblk + b)

                nblk = SC // XCH  # 4 token-blocks per chunk
                for ch in range(XCH):
                    xld = stage.tile([P, nblk, D], BF16, name=f"x{ch}")
                    nc.gpsimd.dma_start(
                        out=xld[:],
                        in_=x_ext.ap().rearrange("(t p) d -> p t d", p=P)[
                            :, ch * nblk : (ch + 1) * nblk, :
                        ],
                    )
                    for b in range(nblk):
                        transpose_block(xld[:, b, :], xT, ch * nblk + b)

            # ---------- phase 2: QKV projections (+ q2 / k2) ----------
            with tc.tile_pool(name="qkv_ps", bufs=4, space="PSUM") as qkv_ps, \
                 tc.tile_pool(name="n2_ps", bufs=2, space="PSUM") as n2_ps, \
                 tc.tile_pool(name="sq_sb", bufs=4) as sq_sb:
                # q and k projections: psum tile [128(f), 512(i)]; squares
                # reduce over f via a [128,1]-weights matmul into [1, 512].
                for which in ("q", "k"):
                    wT = wqT if which == "q" else wkT
                    dstT = qTa if which == "q" else kTa
                    wcol = slice(0, 1) if which == "q" else slice(1, 2)
                    for i4 in range(S // 512):
                        isl = slice(i4 * 512, (i4 + 1) * 512)
                        n2t = n2_ps.tile([1, 512], F32)
                        for fc in range(FC):
                            pp = qkv_ps.tile([P, 512], F32, name="proj")
                            for kc in range(DC):
                                nc.tensor.matmul(
                                    pp[:],
                                    wT[:, kc, fc * P : (fc + 1) * P],
                                    xT[:, kc, isl],
                                    start=(kc == 0),
                                    stop=(kc == DC - 1),
                                )
                            if which == "q":
                                nc.vector.tensor_copy(
                                    out=dstT[:, fc, isl], in_=pp[:]
                                )
                            else:
                                nc.scalar.mul(dstT[:, fc, isl], pp[:], -2.0)
                            sq = sq_sb.tile([P, 512], BF16)
                            nc.scalar.activation(
                                out=sq[:], in_=pp[:], func=AF.Square
                            )
                            nc.tensor.matmul(
                                n2t[:],
                                sumw[:, 0:1],
                                sq[:],
                                start=(fc == 0),
                                stop=(fc == FC - 1),
                            )
                        dstrow = q2row if which == "q" else k2row
                        nc.vector.tensor_scalar_add(
                            dstrow[0:1, isl], n2t[0:1, :], -C2
                        )

                # v projection: psum tile [128(j), n(f)]
                for jt in range(SC):
                    for n0, nw in ((0, 512), (512, F - 512)):
                        vp = qkv_ps.tile([P, 512], F32, name="proj")
                        for kc in range(DC):
                            nc.tensor.matmul(
                                vp[:, 0:nw],
                                xT[:, kc, jt * P : (jt + 1) * P],
                                wvT[:, kc, n0 : n0 + nw],
                                start=(kc == 0),
                                stop=(kc == DC - 1),
                            )
                        nc.scalar.copy(
                            out=vA[:, jt, n0 : n0 + nw], in_=vp[:, 0:nw]
                        )
        # wt pool closed: its zone (compute-written only) hosts o_buf now.

        # ---------- phase 3: distances -> softmax -> attn @ v ----------
        # jt pairs share one [128, 512] psum tile (halves hold two j-blocks
        # over the same 256 i-columns) so DVE/ACT run at 512-wide.
        with tc.tile_pool(name="o_pool", bufs=1) as o_pool, \
             tc.tile_pool(name="out_ps", bufs=1, space="PSUM") as out_ps_pool, \
             tc.tile_pool(name="d2_ps", bufs=3, space="PSUM") as d2_ps, \
             tc.tile_pool(name="m_sb", bufs=3) as m_sb, \
             tc.tile_pool(name="s2_sb", bufs=3) as s2_sb, \
             tc.tile_pool(name="p_sb", bufs=3) as p_sb, \
             tc.tile_pool(name="r_sb", bufs=3) as r_sb:
            o_buf = o_pool.tile([P, SC, F], BF16)
            for ig in range(NIG):
                igsl = slice(ig * IG, (ig + 1) * IG)
                outp = [
                    out_ps_pool.tile([P, FV], F32, name=f"outp{ic}")
                    for ic in range(NI)
                ]
                for jp in range(SC // 2):
                    d2 = d2_ps.tile([P, 2 * IG], F32)
                    for half in range(2):
                        jt = 2 * jp + half
                        jsl = slice(jt * P, (jt + 1) * P)
                        dsl = slice(half * IG, (half + 1) * IG)
                        for c in range(DC // 2):
                            nc.tensor.matmul(
                                d2[:, dsl],
                                kTa[:, 2 * c : 2 * c + 2, jsl],
                                qTa[:, 2 * c : 2 * c + 2, igsl],
                                start=(c == 0),
                                stop=False,
                                perf_mode=mybir.MatmulPerfMode.DoubleRow,
                            )
                        # k2 row:  (k2-768)[j] x ones[i]
                        nc.tensor.matmul(
                            d2[:, dsl],
                            k2row[0:1, jsl],
                            ones_row[0:1, igsl],
                            start=False,
                            stop=False,
                        )
                        # q2 row:  ones[j] x (q2-768)[i]
                        nc.tensor.matmul(
                            d2[:, dsl],
                            ones_row[0:1, jsl],
                            q2row[0:1, igsl],
                            start=False,
                            stop=True,
                        )
                    m = m_sb.tile([P, 2 * IG], F32)
                    nc.vector.tensor_scalar(
                        m[:], d2[:], ALPHA, BETA, op0=ALU.mult, op1=ALU.add
                    )
                    s2 = s2_sb.tile([P, 2 * IG], F32)
                    nc.vector.tensor_mul(s2[:], m[:], m[:])
                    p = p_sb.tile([P, 2 * IG], BF16)
                    nc.scalar.activation(
                        out=p[:], in_=s2[:], func=AF.Exp, scale=-1.0
                    )
                    for half in range(2):
                        jt = 2 * jp + half
                        for ic in range(NI):
                            lhs = p[
                                :, half * IG + ic * P : half * IG + (ic + 1) * P
                            ]
                            for n0, nw in ((0, 512), (512, FV - 512)):
                                nc.tensor.matmul(
                                    outp[ic][:, n0 : n0 + nw],
                                    lhs,
                                    vA[:, jt, n0 : n0 + nw],
                                    start=(jt == 0),
                                    stop=(jt == SC - 1),
                                )
                for ic in range(NI):
                    tidx = ig * NI + ic
                    rcp = r_sb.tile([P, 1], F32)
                    nc.vector.reciprocal(rcp[:], outp[ic][:, F : F + 1])
                    nc.vector.tensor_scalar_mul(
                        o_buf[:, tidx, :], outp[ic][:, 0:F], rcp[:]
                    )
                    i0 = tidx * P
                    nc.gpsimd.dma_start(
                        out=out_ext[i0 : i0 + P, :], in_=o_buf[:, tidx, :]
                    )

    return nc


_DMA_TYPES = (
    "InstDMACopy",
    "InstDmaTransposeAnt",
    "InstDMA",
    "InstDMAGatherAnt",
    "InstDMAScatterAddAnt",
)
_SKIP_TYPES = (
    "InstUnconditionalBranch",
    "InstCall",
    "InstISA",
    "InstCompareAndBranch",
    "InstIndirectBranch",
    "InstHalt",
)


def _split_dma_waits(nc: bass.Bass) -> bass.Bass:
    """walrus' setupSyncWait accepts a limited number of embedded semaphore
    waits per instruction (1 for DMA, 2 for regular TPB instructions);
    hoist any extra waits into NoOps on the issuing engine (engine FIFO
    order makes them complete before the instruction issues)."""
    ctr = 0
    for bb in nc.m.functions[0].blocks:
        new_insts = []
        for ins in bb.instructions:
            t = type(ins).__name__
            si = ins.sync_info
            cap = 1
            if t not in _SKIP_TYPES and si is not None and len(si.on_wait) > cap:
                for w in si.on_wait[:-cap]:
                    ctr += 1
                    new_insts.append(
                        mybir.InstNoOp(
                            name=f"antwait-{ctr}",
                            engine=ins.engine,
                            sync_info=mybir.SyncInfo(on_wait=[w], on_update=[]),
                        )
                    )
                ins.sync_info = mybir.SyncInfo(
                    on_wait=list(si.on_wait[-cap:]), on_update=list(si.on_update)
                )
            new_insts.append(ins)
        bb.instructions = new_insts
    return nc


_CACHED_NC = None


def _get_nc():
    global _CACHED_NC
    if _CACHED_NC is None:
        _CACHED_NC = _split_dma_waits(build_bass())
    return _CACHED_NC


def _bf16(a):
    return np.asarray(a, dtype=np.float32).astype(ml_dtypes.bfloat16)


def kernel(x, Wq, Wk, Wv, _trace=False):
    x = _bf16(x)
    Wq = np.ascontiguousarray(_bf16(Wq))
    Wk = np.ascontiguousarray(_bf16(Wk))
    Wv = np.ascontiguousarray(_bf16(Wv))
    assert x.shape == (N_CORES, S, D)

    nc = _get_nc()
    in_maps = [
        {"x": np.ascontiguousarray(x[b]), "Wq": Wq, "Wk": Wk, "Wv": Wv}
        for b in range(N_CORES)
    ]
    res = run_bass_kernel_spmd(
        nc, in_maps, core_ids=list(range(N_CORES)), trace=_trace
    )
    out = np.stack(
        [res.results[b]["out"].astype(np.float32) for b in range(N_CORES)],
        axis=0,
    )
    if _trace:
        return out, res
    return out


# revision 42
# speedup vs baseline: 1.2625x; 1.0623x over previous
"""Distance-attention kernel for Trainium2, SPMD over 8 NeuronCores.

Reference computation (per batch element b):
    q = x @ Wq.T ; k = x @ Wk.T ; v = x @ Wv.T          [S, F]
    scores = cdist(q, k) / sqrt(768)                     [S, S]
    attn = softmax(scores)   (softmax of RAW distances)
    out = attn @ v                                       [S, F]

Sharding: data-parallel over batch. B == 8 == n_cores, so core b computes
batch element b end-to-end; weights are replicated. No collectives.

Device algorithm (matmul inputs bf16, fp32 PSUM accumulation):

  d2[j, i] - 1536 = (k2[j]-768) + (q2[i]-768) - 2*qk[j, i] comes from ONE
  augmented GEMM: K-chunks 0..5 hold (-2*k)^T against q^T; two K=1 chunks
  add the norm rows: (k2-768)[j] x ones[i] and ones[j] x (q2-768)[i].
  All four aug operands live on partition 0 of their tensors (engines
  cannot address partition >0 as an op start, and DMA instructions only
  support a single semaphore wait in this toolchain, so no partition-1
  staging DMAs are possible).  Centering by 768 (~E[q2]) keeps the
  bf16-stored aug rows accurate.

  scores = sqrt(d2/768) is needed only inside a softmax, so any additive
  constant cancels.  Over the realizable d2 range ([~1060, ~2160]; fit on
  [900, 2300]) a minimax quadratic  s ~ a*(w+h)^2 + C  (w = d2-1536,
  a < 0) is accurate to 4e-3, below bf16 logit noise.  Dropping C:
      p = exp(s - C) = exp(-(alpha*w + beta)^2)
  -> one fused DVE tensor_scalar (m = alpha*w + beta), one DVE square,
  one ACT Exp(scale=-1).  No Sqrt on ACT at all, so the ACT LUT never
  switches table sets inside the hot loop (a Sqrt/Exp alternation would
  cost ~5.3us per iteration in table loads).

  attn@v and the softmax denominator come from ONE GEMM against v
  augmented with a ones column: out_psum[i, 0:768] = sum_j p*v,
  out_psum[i, 768] = sum_j p.  Epilogue: out = out_psum[:, :768] *
  (1/out_psum[:, 768]), written bf16 (host upcasts to f32).

DMA discipline (walrus rejects DMA instructions with >1 semaphore wait):
inputs arrive pre-cast to bf16 (no on-device cast producer), every load
DMA targets a fresh never-reused tile in a fresh SBUF zone (zero waits),
and stores read a persistent staging buffer placed in the zone vacated by
the weight-transpose pool (compute-only history => the store's only wait
is its DVE producer).
"""

import numpy as np
import ml_dtypes
from contextlib import ExitStack

import concourse.bass as bass
import concourse.tile as tile
from concourse import mybir
from concourse.bass_utils import run_bass_kernel_spmd
from concourse.masks import make_identity

S, D, F = 2048, 768, 768
P = 128
SC = S // P          # 16 token chunks
DC = D // P          # 6 contraction chunks for QKV
FC = F // P          # 6
IG = 256             # i-group width in the main loop
NIG = S // IG        # 8
NI = IG // P         # 2 i-chunks per group
FV = F + 1           # v columns + ones column
C2 = 768.0           # centering constant for q2/k2 rows
BF16 = mybir.dt.bfloat16
FP8 = mybir.dt.float8e4
F32 = mybir.dt.float32
AF = mybir.ActivationFunctionType
ALU = mybir.AluOpType

# minimax quadratic for sqrt((w+1536)/768), w = d2-1536 in [900,2300]-1536:
# s ~ -(ALPHA*w + BETA)^2 + C  =>  softmax weights p = exp(-(ALPHA*w+BETA)^2)
ALPHA = 1.929106785463655e-04
BETA = -1.2116557543348982

N_CORES = 8

XCH = 4              # x loaded in 4 chunks of 4 token-blocks
WCH = 3              # each W loaded in 3 chunks of 2 row-blocks


def build_bass(target_bir_lowering: bool = False) -> bass.Bass:
    nc = bass.Bass(target_bir_lowering=target_bir_lowering)
    x_ext = nc.declare_dram_parameter("x", [S, D], BF16, isOutput=False)
    wq_ext = nc.declare_dram_parameter("Wq", [F, D], BF16, isOutput=False)
    wk_ext = nc.declare_dram_parameter("Wk", [F, D], BF16, isOutput=False)
    wv_ext = nc.declare_dram_parameter("Wv", [F, D], BF16, isOutput=False)
    out_ext = nc.declare_dram_parameter("out", [S, F], BF16, isOutput=True)

    with tile.TileContext(nc) as tc, ExitStack() as ctx:
        big = ctx.enter_context(tc.tile_pool(name="big", bufs=1))
        xT = big.tile([P, DC, S], BF16)       # x^T   [d, i] (chunked d)
        xT8 = big.tile([P, DC, S], FP8)       # x^T fp8 copy for q/k GEMMs
        qTa = big.tile([P, DC, S], FP8)       # q^T  (fp8 for DoubleRow)
        kTa = big.tile([P, DC, S], FP8)       # -2*k^T (fp8 for DoubleRow)
        vA = big.tile([P, SC, FV], BF16)      # v (+ ones col)  [j, f+1]
        # sum weights for the norm reduction matmul: col 0 -> 1.0 (q2 from
        # q^2), col 1 -> 0.25 (k2 from (-2k)^2)
        sumw = big.tile([P, 2], BF16)
        q2row = big.tile([1, S], BF16)        # (q2-768)[i] on partition 0
        k2row = big.tile([1, S], BF16)        # (k2-768)[j] on partition 0
        ones_row = big.tile([1, S], BF16)     # ones on partition 0
        ident = big.tile([P, P], BF16)
        make_identity(nc, ident[:])

        nc.vector.memset(sumw[:, 0:1], 1.0)
        nc.vector.memset(sumw[:, 1:2], 0.25)
        nc.vector.memset(ones_row[:], 1.0)
        nc.vector.memset(vA[:, :, F : F + 1], 1.0)

        # weight transposes live only through phase 2; phase 3 reuses their
        # SBUF zone for the output staging buffer.
        wt_pool = tc.tile_pool(name="wt", bufs=1)
        with wt_pool as wt:
            wqT = wt.tile([P, DC, F], FP8)
            wkT = wt.tile([P, DC, F], FP8)
            wvT = wt.tile([P, DC, F], BF16)
            # pad the pool so its zone still covers o_buf (24KB) in phase 3
            wt_pad = wt.tile([P, 3072], BF16)

            # ---------- phase 1: load (bf16 from host), PE-transpose ------
            with tc.tile_pool(name="stage", bufs=1) as stage, \
                 tc.tile_pool(name="tr_ps", bufs=2, space="PSUM") as tr_ps:

                def transpose_block(src, dst, t):
                    # src: [P, D] natural bf16 -> dst[:, c, t*P:(t+1)*P]
                    for c in range(DC):
                        tp = tr_ps.tile([P, P], BF16, name="tp")
                        nc.tensor.transpose(
                            tp[:], src[:, c * P : (c + 1) * P], ident[:]
                        )
                        cp = (
                            nc.scalar.copy
                            if c % 2 == 0
                            else nc.vector.tensor_copy
                        )
                        cp(out=dst[:, c, t * P : (t + 1) * P], in_=tp[:])

                for ext, dst, wname in (
                    (wq_ext, wqT, "wq"),
                    (wk_ext, wkT, "wk"),
                    (wv_ext, wvT, "wv"),
                ):
                    nblk = FC // WCH  # 2 row-blocks per chunk
                    for ch in range(WCH):
                        wld = stage.tile(
                            [P, nblk, D], BF16, name=f"{wname}{ch}"
                        )
                        nc.gpsimd.dma_start(
                            out=wld[:],
                            in_=ext.ap().rearrange("(c p) d -> p c d", p=P)[
                                :, ch * nblk : (ch + 1) * nblk, :
                            ],
                        )
                        for b in range(nblk):
                            transpose_block(wld[:, b, :], dst, ch * nblk + b)

                nblk = SC // XCH  # 4 token-blocks per chunk
                for ch in range(XCH):
                    xld = stage.tile([P, nblk, D], BF16, name=f"x{ch}")
                    nc.gpsimd.dma_start(
                        out=xld[:],
                        in_=x_ext.ap().rearrange("(t p) d -> p t d", p=P)[
                            :, ch * nblk : (ch + 1) * nblk, :
                        ],
                    )
                    for b in range(nblk):
                        transpose_block(xld[:, b, :], xT, ch * nblk + b)
                for c in range(DC):
                    nc.vector.tensor_copy(out=xT8[:, c, :], in_=xT[:, c, :])

            # ---------- phase 2: QKV projections (+ q2 / k2) ----------
            with tc.tile_pool(name="qkv_ps", bufs=4, space="PSUM") as qkv_ps, \
                 tc.tile_pool(name="n2_ps", bufs=2, space="PSUM") as n2_ps, \
                 tc.tile_pool(name="sq_sb", bufs=4) as sq_sb:
                # q and k projections: psum tile [128(f), 512(i)]; squares
                # reduce over f via a [128,1]-weights matmul into [1, 512].
                for which in ("q", "k"):
                    wT = wqT if which == "q" else wkT
                    dstT = qTa if which == "q" else kTa
                    wcol = slice(0, 1) if which == "q" else slice(1, 2)
                    for i4 in range(S // 512):
                        isl = slice(i4 * 512, (i4 + 1) * 512)
                        n2t = n2_ps.tile([1, 512], F32)
                        for fc in range(FC):
                            pp = qkv_ps.tile([P, 512], F32, name="proj")
                            for c in range(DC // 2):
                                nc.tensor.matmul(
                                    pp[:],
                                    wT[:, 2 * c : 2 * c + 2, fc * P : (fc + 1) * P],
                                    xT8[:, 2 * c : 2 * c + 2, isl],
                                    start=(c == 0),
                                    stop=(c == DC // 2 - 1),
                                    perf_mode=mybir.MatmulPerfMode.DoubleRow,
                                )
                            if which == "q":
                                nc.vector.tensor_copy(
                                    out=dstT[:, fc, isl], in_=pp[:]
                                )
                            else:
                                nc.scalar.mul(dstT[:, fc, isl], pp[:], -2.0)
                            sq = sq_sb.tile([P, 512], BF16)
                            nc.scalar.activation(
                                out=sq[:], in_=pp[:], func=AF.Square
                            )
                            nc.tensor.matmul(
                                n2t[:],
                                sumw[:, 0:1],
                                sq[:],
                                start=(fc == 0),
                                stop=(fc == FC - 1),
                            )
                        dstrow = q2row if which == "q" else k2row
                        nc.vector.tensor_scalar_add(
                            dstrow[0:1, isl], n2t[0:1, :], -C2
                        )

                # v projection: psum tile [128(j), n(f)]
                for jt in range(SC):
                    for n0, nw in ((0, 512), (512, F - 512)):
                        vp = qkv_ps.tile([P, 512], F32, name="proj")
                        for kc in range(DC):
                            nc.tensor.matmul(
                                vp[:, 0:nw],
                                xT[:, kc, jt * P : (jt + 1) * P],
                                wvT[:, kc, n0 : n0 + nw],
                                start=(kc == 0),
                                stop=(kc == DC - 1),
                            )
                        nc.scalar.copy(
                            out=vA[:, jt, n0 : n0 + nw], in_=vp[:, 0:nw]
                        )
        # wt pool closed: its zone (compute-written only) hosts o_buf now.

        # ---------- phase 3: distances -> softmax -> attn @ v ----------
        # jt pairs share one [128, 512] psum tile (halves hold two j-blocks
        # over the same 256 i-columns) so DVE/ACT run at 512-wide.
        with tc.tile_pool(name="o_pool", bufs=1) as o_pool, \
             tc.tile_pool(name="out_ps", bufs=1, space="PSUM") as out_ps_pool, \
             tc.tile_pool(name="d2_ps", bufs=3, space="PSUM") as d2_ps, \
             tc.tile_pool(name="m_sb", bufs=3) as m_sb, \
             tc.tile_pool(name="s2_sb", bufs=3) as s2_sb, \
             tc.tile_pool(name="p_sb", bufs=3) as p_sb, \
             tc.tile_pool(name="r_sb", bufs=3) as r_sb:
            o_buf = o_pool.tile([P, SC, F], BF16)
            for ig in range(NIG):
                igsl = slice(ig * IG, (ig + 1) * IG)
                outp = [
                    out_ps_pool.tile([P, FV], F32, name=f"outp{ic}")
                    for ic in range(NI)
                ]
                for jp in range(SC // 2):
                    d2 = d2_ps.tile([P, 2 * IG], F32)
                    for half in range(2):
                        jt = 2 * jp + half
                        jsl = slice(jt * P, (jt + 1) * P)
                        dsl = slice(half * IG, (half + 1) * IG)
                        for c in range(DC // 2):
                            nc.tensor.matmul(
                                d2[:, dsl],
                                kTa[:, 2 * c : 2 * c + 2, jsl],
                                qTa[:, 2 * c : 2 * c + 2, igsl],
                                start=(c == 0),
                                stop=False,
                                perf_mode=mybir.MatmulPerfMode.DoubleRow,
                            )
                        # k2 row:  (k2-768)[j] x ones[i]
                        nc.tensor.matmul(
                            d2[:, dsl],
                            k2row[0:1, jsl],
                            ones_row[0:1, igsl],
                            start=False,
                            stop=False,
                        )
                        # q2 row:  ones[j] x (q2-768)[i]
                        nc.tensor.matmul(
                            d2[:, dsl],
                            ones_row[0:1, jsl],
                            q2row[0:1, igsl],
                            start=False,
                            stop=True,
                        )
                    m = m_sb.tile([P, 2 * IG], F32)
                    nc.vector.tensor_scalar(
                        m[:], d2[:], ALPHA, BETA, op0=ALU.mult, op1=ALU.add
                    )
                    s2 = s2_sb.tile([P, 2 * IG], F32)
                    nc.vector.tensor_mul(s2[:], m[:], m[:])
                    p = p_sb.tile([P, 2 * IG], BF16)
                    nc.scalar.activation(
                        out=p[:], in_=s2[:], func=AF.Exp, scale=-1.0
                    )
                    for half in range(2):
                        jt = 2 * jp + half
                        for ic in range(NI):
                            lhs = p[
                                :, half * IG + ic * P : half * IG + (ic + 1) * P
                            ]
                            for n0, nw in ((0, 512), (512, FV - 512)):
                                nc.tensor.matmul(
                                    outp[ic][:, n0 : n0 + nw],
                                    lhs,
                                    vA[:, jt, n0 : n0 + nw],
                                    start=(jt == 0),
                                    stop=(jt == SC - 1),
                                )
                for ic in range(NI):
                    tidx = ig * NI + ic
                    rcp = r_sb.tile([P, 1], F32)
                    nc.vector.reciprocal(rcp[:], outp[ic][:, F : F + 1])
                    nc.vector.tensor_scalar_mul(
                        o_buf[:, tidx, :], outp[ic][:, 0:F], rcp[:]
                    )
                    i0 = tidx * P
                    nc.gpsimd.dma_start(
                        out=out_ext[i0 : i0 + P, :], in_=o_buf[:, tidx, :]
                    )

    return nc


_DMA_TYPES = (
    "InstDMACopy",
    "InstDmaTransposeAnt",
    "InstDMA",
    "InstDMAGatherAnt",
    "InstDMAScatterAddAnt",
)
_SKIP_TYPES = (
    "InstUnconditionalBranch",
    "InstCall",
    "InstISA",
    "InstCompareAndBranch",
    "InstIndirectBranch",
    "InstHalt",
)


def _split_dma_waits(nc: bass.Bass) -> bass.Bass:
    """walrus' setupSyncWait accepts a limited number of embedded semaphore
    waits per instruction (1 for DMA, 2 for regular TPB instructions);
    hoist any extra waits into NoOps on the issuing engine (engine FIFO
    order makes them complete before the instruction issues)."""
    ctr = 0
    for bb in nc.m.functions[0].blocks:
        new_insts = []
        for ins in bb.instructions:
            t = type(ins).__name__
            si = ins.sync_info
            cap = 1
            if t not in _SKIP_TYPES and si is not None and len(si.on_wait) > cap:
                for w in si.on_wait[:-cap]:
                    ctr += 1
                    new_insts.append(
                        mybir.InstNoOp(
                            name=f"antwait-{ctr}",
                            engine=ins.engine,
                            sync_info=mybir.SyncInfo(on_wait=[w], on_update=[]),
                        )
                    )
                ins.sync_info = mybir.SyncInfo(
                    on_wait=list(si.on_wait[-cap:]), on_update=list(si.on_update)
                )
            new_insts.append(ins)
        bb.instructions = new_insts
    return nc


_CACHED_NC = None


def _get_nc():
    global _CACHED_NC
    if _CACHED_NC is None:
        _CACHED_NC = _split_dma_waits(build_bass())
    return _CACHED_NC


def _bf16(a):
    return np.asarray(a, dtype=np.float32).astype(ml_dtypes.bfloat16)


def kernel(x, Wq, Wk, Wv, _trace=False):
    x = _bf16(x)
    Wq = np.ascontiguousarray(_bf16(Wq))
    Wk = np.ascontiguousarray(_bf16(Wk))
    Wv = np.ascontiguousarray(_bf16(Wv))
    assert x.shape == (N_CORES, S, D)

    nc = _get_nc()
    in_maps = [
        {"x": np.ascontiguousarray(x[b]), "Wq": Wq, "Wk": Wk, "Wv": Wv}
        for b in range(N_CORES)
    ]
    res = run_bass_kernel_spmd(
        nc, in_maps, core_ids=list(range(N_CORES)), trace=_trace
    )
    out = np.stack(
        [res.results[b]["out"].astype(np.float32) for b in range(N_CORES)],
        axis=0,
    )
    if _trace:
        return out, res
    return out


# revision 43
# speedup vs baseline: 1.3607x; 1.0777x over previous
"""Distance-attention kernel for Trainium2, SPMD over 8 NeuronCores.

Reference computation (per batch element b):
    q = x @ Wq.T ; k = x @ Wk.T ; v = x @ Wv.T          [S, F]
    scores = cdist(q, k) / sqrt(768)                     [S, S]
    attn = softmax(scores)   (softmax of RAW distances)
    out = attn @ v                                       [S, F]

Sharding: data-parallel over batch. B == 8 == n_cores, so core b computes
batch element b end-to-end; weights are replicated. No collectives.

Device algorithm (matmul inputs bf16, fp32 PSUM accumulation):

  d2[j, i] - 1536 = (k2[j]-768) + (q2[i]-768) - 2*qk[j, i] comes from ONE
  augmented GEMM: K-chunks 0..5 hold (-2*k)^T against q^T; two K=1 chunks
  add the norm rows: (k2-768)[j] x ones[i] and ones[j] x (q2-768)[i].
  All four aug operands live on partition 0 of their tensors (engines
  cannot address partition >0 as an op start, and DMA instructions only
  support a single semaphore wait in this toolchain, so no partition-1
  staging DMAs are possible).  Centering by 768 (~E[q2]) keeps the
  bf16-stored aug rows accurate.

  scores = sqrt(d2/768) is needed only inside a softmax, so any additive
  constant cancels.  Over the realizable d2 range ([~1060, ~2160]; fit on
  [900, 2300]) a minimax quadratic  s ~ a*(w+h)^2 + C  (w = d2-1536,
  a < 0) is accurate to 4e-3, below bf16 logit noise.  Dropping C:
      p = exp(s - C) = exp(-(alpha*w + beta)^2)
  -> one fused DVE tensor_scalar (m = alpha*w + beta), one DVE square,
  one ACT Exp(scale=-1).  No Sqrt on ACT at all, so the ACT LUT never
  switches table sets inside the hot loop (a Sqrt/Exp alternation would
  cost ~5.3us per iteration in table loads).

  attn@v and the softmax denominator come from ONE GEMM against v
  augmented with a ones column: out_psum[i, 0:768] = sum_j p*v,
  out_psum[i, 768] = sum_j p.  Epilogue: out = out_psum[:, :768] *
  (1/out_psum[:, 768]), written bf16 (host upcasts to f32).

DMA discipline (walrus rejects DMA instructions with >1 semaphore wait):
inputs arrive pre-cast to bf16 (no on-device cast producer), every load
DMA targets a fresh never-reused tile in a fresh SBUF zone (zero waits),
and stores read a persistent staging buffer placed in the zone vacated by
the weight-transpose pool (compute-only history => the store's only wait
is its DVE producer).
"""

import numpy as np
import ml_dtypes
from contextlib import ExitStack

import concourse.bass as bass
import concourse.tile as tile
from concourse import mybir
from concourse.bass_utils import run_bass_kernel_spmd
from concourse.masks import make_identity

S, D, F = 2048, 768, 768
P = 128
SC = S // P          # 16 token chunks
DC = D // P          # 6 contraction chunks for QKV
FC = F // P          # 6
IG = 256             # i-group width in the main loop
NIG = S // IG        # 8
NI = IG // P         # 2 i-chunks per group
FV = F + 1           # v columns + ones column
C2 = 768.0           # centering constant for q2/k2 rows
BF16 = mybir.dt.bfloat16
FP8 = mybir.dt.float8e4
F32 = mybir.dt.float32
AF = mybir.ActivationFunctionType
ALU = mybir.AluOpType

# minimax quadratic for sqrt((w+1536)/768), w = d2-1536 in [900,2300]-1536:
# s ~ -(ALPHA*w + BETA)^2 + C  =>  softmax weights p = exp(-(ALPHA*w+BETA)^2)
ALPHA = 1.929106785463655e-04
BETA = -1.2116557543348982

N_CORES = 8

XCH = 4              # x loaded in 4 chunks of 4 token-blocks
WCH = 3              # each W loaded in 3 chunks of 2 row-blocks


def build_bass(target_bir_lowering: bool = False) -> bass.Bass:
    nc = bass.Bass(target_bir_lowering=target_bir_lowering)
    x_ext = nc.declare_dram_parameter("x", [S, D], BF16, isOutput=False)
    wq_ext = nc.declare_dram_parameter("Wq", [F, D], BF16, isOutput=False)
    wk_ext = nc.declare_dram_parameter("Wk", [F, D], BF16, isOutput=False)
    wv_ext = nc.declare_dram_parameter("Wv", [F, D], BF16, isOutput=False)
    out_ext = nc.declare_dram_parameter("out", [S, F], BF16, isOutput=True)

    with tile.TileContext(nc) as tc, ExitStack() as ctx:
        big = ctx.enter_context(tc.tile_pool(name="big", bufs=1))
        xT = big.tile([P, DC, S], BF16)       # x^T   [d, i] (chunked d)
        xT8 = big.tile([P, DC, S], FP8)       # x^T fp8 copy for q/k GEMMs
        qTa = big.tile([P, DC, S], FP8)       # q^T  (fp8 for DoubleRow)
        kTa = big.tile([P, DC, S], FP8)       # -2*k^T (fp8 for DoubleRow)
        vA = big.tile([P, SC, FV], BF16)      # v (+ ones col)  [j, f+1]
        # sum weights for the norm reduction matmul: col 0 -> 1.0 (q2 from
        # q^2), col 1 -> 0.25 (k2 from (-2k)^2)
        sumw = big.tile([P, 2], BF16)
        q2row = big.tile([1, S], BF16)        # (q2-768)[i] on partition 0
        k2row = big.tile([1, S], BF16)        # (k2-768)[j] on partition 0
        ones_row = big.tile([1, S], BF16)     # ones on partition 0
        ident = big.tile([P, P], BF16)
        make_identity(nc, ident[:])

        nc.vector.memset(sumw[:, 0:1], 1.0)
        nc.vector.memset(sumw[:, 1:2], 0.25)
        nc.vector.memset(ones_row[:], 1.0)
        nc.vector.memset(vA[:, :, F : F + 1], 1.0)

        # weight transposes live only through phase 2; phase 3 reuses their
        # SBUF zone for the output staging buffer.
        wt_pool = tc.tile_pool(name="wt", bufs=1)
        with wt_pool as wt:
            wqT = wt.tile([P, DC, F], FP8)
            wkT = wt.tile([P, DC, F], FP8)
            wvT = wt.tile([P, DC, F], BF16)
            # pad the pool so its zone still covers o_buf (24KB) in phase 3
            wt_pad = wt.tile([P, 3072], BF16)

            # ---------- phase 1: load (bf16 from host), PE-transpose ------
            with tc.tile_pool(name="stage", bufs=1) as stage, \
                 tc.tile_pool(name="tr_ps", bufs=3, space="PSUM") as tr_ps:

                def transpose_block(src, dst, t):
                    # src: [P, D] natural bf16 -> dst[:, c, t*P:(t+1)*P]
                    for c in range(DC):
                        tp = tr_ps.tile([P, P], BF16, name="tp")
                        nc.tensor.transpose(
                            tp[:], src[:, c * P : (c + 1) * P], ident[:]
                        )
                        cp = (
                            nc.scalar.copy
                            if c % 2 == 0
                            else nc.vector.tensor_copy
                        )
                        cp(out=dst[:, c, t * P : (t + 1) * P], in_=tp[:])

                for ext, dst, wname in (
                    (wq_ext, wqT, "wq"),
                    (wk_ext, wkT, "wk"),
                    (wv_ext, wvT, "wv"),
                ):
                    nblk = FC // WCH  # 2 row-blocks per chunk
                    for ch in range(WCH):
                        wld = stage.tile(
                            [P, nblk, D], BF16, name=f"{wname}{ch}"
                        )
                        nc.sync.dma_start(
                            out=wld[:],
                            in_=ext.ap().rearrange("(c p) d -> p c d", p=P)[
                                :, ch * nblk : (ch + 1) * nblk, :
                            ],
                        )
                        for b in range(nblk):
                            transpose_block(wld[:, b, :], dst, ch * nblk + b)

                nblk = SC // XCH  # 4 token-blocks per chunk
                for ch in range(XCH):
                    xld = stage.tile([P, nblk, D], BF16, name=f"x{ch}")
                    nc.sync.dma_start(
                        out=xld[:],
                        in_=x_ext.ap().rearrange("(t p) d -> p t d", p=P)[
                            :, ch * nblk : (ch + 1) * nblk, :
                        ],
                    )
                    for b in range(nblk):
                        transpose_block(xld[:, b, :], xT, ch * nblk + b)
                for c in range(DC):
                    nc.vector.tensor_copy(out=xT8[:, c, :], in_=xT[:, c, :])

            # ---------- phase 2: QKV projections (+ q2 / k2) ----------
            with tc.tile_pool(name="qkv_ps", bufs=4, space="PSUM") as qkv_ps, \
                 tc.tile_pool(name="n2_ps", bufs=2, space="PSUM") as n2_ps, \
                 tc.tile_pool(name="sq_sb", bufs=4) as sq_sb:
                # q and k projections: psum tile [128(f), 512(i)]; squares
                # reduce over f via a [128,1]-weights matmul into [1, 512].
                for which in ("q", "k"):
                    wT = wqT if which == "q" else wkT
                    dstT = qTa if which == "q" else kTa
                    wcol = slice(0, 1) if which == "q" else slice(1, 2)
                    for i4 in range(S // 512):
                        isl = slice(i4 * 512, (i4 + 1) * 512)
                        n2t = n2_ps.tile([1, 512], F32)
                        for fc in range(FC):
                            pp = qkv_ps.tile([P, 512], F32, name="proj")
                            for c in range(DC // 2):
                                nc.tensor.matmul(
                                    pp[:],
                                    wT[:, 2 * c : 2 * c + 2, fc * P : (fc + 1) * P],
                                    xT8[:, 2 * c : 2 * c + 2, isl],
                                    start=(c == 0),
                                    stop=(c == DC // 2 - 1),
                                    perf_mode=mybir.MatmulPerfMode.DoubleRow,
                                )
                            if which == "q":
                                nc.vector.tensor_copy(
                                    out=dstT[:, fc, isl], in_=pp[:]
                                )
                            else:
                                nc.scalar.mul(dstT[:, fc, isl], pp[:], -2.0)
                            sq = sq_sb.tile([P, 512], BF16)
                            nc.scalar.activation(
                                out=sq[:], in_=pp[:], func=AF.Square
                            )
                            nc.tensor.matmul(
                                n2t[:],
                                sumw[:, 0:1],
                                sq[:],
                                start=(fc == 0),
                                stop=(fc == FC - 1),
                            )
                        dstrow = q2row if which == "q" else k2row
                        nc.vector.tensor_scalar_add(
                            dstrow[0:1, isl], n2t[0:1, :], -C2
                        )

                # v projection: psum tile [128(j), n(f)]
                for jt in range(SC):
                    for n0, nw in ((0, 512), (512, F - 512)):
                        vp = qkv_ps.tile([P, 512], F32, name="proj")
                        for kc in range(DC):
                            nc.tensor.matmul(
                                vp[:, 0:nw],
                                xT[:, kc, jt * P : (jt + 1) * P],
                                wvT[:, kc, n0 : n0 + nw],
                                start=(kc == 0),
                                stop=(kc == DC - 1),
                            )
                        nc.scalar.copy(
                            out=vA[:, jt, n0 : n0 + nw], in_=vp[:, 0:nw]
                        )
        # wt pool closed: its zone (compute-written only) hosts o_buf now.

        # ---------- phase 3: distances -> softmax -> attn @ v ----------
        # jt pairs share one [128, 512] psum tile (halves hold two j-blocks
        # over the same 256 i-columns) so DVE/ACT run at 512-wide.
        with tc.tile_pool(name="o_pool", bufs=1) as o_pool, \
             tc.tile_pool(name="out_ps", bufs=1, space="PSUM") as out_ps_pool, \
             tc.tile_pool(name="d2_ps", bufs=3, space="PSUM") as d2_ps, \
             tc.tile_pool(name="m_sb", bufs=3) as m_sb, \
             tc.tile_pool(name="s2_sb", bufs=3) as s2_sb, \
             tc.tile_pool(name="p_sb", bufs=3) as p_sb, \
             tc.tile_pool(name="r_sb", bufs=3) as r_sb:
            o_buf = o_pool.tile([P, SC, F], BF16)
            for ig in range(NIG):
                igsl = slice(ig * IG, (ig + 1) * IG)
                outp = [
                    out_ps_pool.tile([P, FV], F32, name=f"outp{ic}")
                    for ic in range(NI)
                ]
                for jp in range(SC // 2):
                    d2 = d2_ps.tile([P, 2 * IG], F32)
                    for half in range(2):
                        jt = 2 * jp + half
                        jsl = slice(jt * P, (jt + 1) * P)
                        dsl = slice(half * IG, (half + 1) * IG)
                        for c in range(DC // 2):
                            nc.tensor.matmul(
                                d2[:, dsl],
                                kTa[:, 2 * c : 2 * c + 2, jsl],
                                qTa[:, 2 * c : 2 * c + 2, igsl],
                                start=(c == 0),
                                stop=False,
                                perf_mode=mybir.MatmulPerfMode.DoubleRow,
                            )
                        # k2 row:  (k2-768)[j] x ones[i]
                        nc.tensor.matmul(
                            d2[:, dsl],
                            k2row[0:1, jsl],
                            ones_row[0:1, igsl],
                            start=False,
                            stop=False,
                        )
                        # q2 row:  ones[j] x (q2-768)[i]
                        nc.tensor.matmul(
                            d2[:, dsl],
                            ones_row[0:1, jsl],
                            q2row[0:1, igsl],
                            start=False,
                            stop=True,
                        )
                    m = m_sb.tile([P, 2 * IG], F32)
                    nc.vector.tensor_scalar(
                        m[:], d2[:], ALPHA, BETA, op0=ALU.mult, op1=ALU.add
                    )
                    s2 = s2_sb.tile([P, 2 * IG], F32)
                    nc.vector.tensor_mul(s2[:], m[:], m[:])
                    p = p_sb.tile([P, 2 * IG], BF16)
                    nc.scalar.activation(
                        out=p[:], in_=s2[:], func=AF.Exp, scale=-1.0
                    )
                    for half in range(2):
                        jt = 2 * jp + half
                        for ic in range(NI):
                            lhs = p[
                                :, half * IG + ic * P : half * IG + (ic + 1) * P
                            ]
                            for n0, nw in ((0, 512), (512, FV - 512)):
                                nc.tensor.matmul(
                                    outp[ic][:, n0 : n0 + nw],
                                    lhs,
                                    vA[:, jt, n0 : n0 + nw],
                                    start=(jt == 0),
                                    stop=(jt == SC - 1),
                                )
                for ic in range(NI):
                    tidx = ig * NI + ic
                    rcp = r_sb.tile([P, 1], F32)
                    nc.vector.reciprocal(rcp[:], outp[ic][:, F : F + 1])
                    nc.vector.tensor_scalar_mul(
                        o_buf[:, tidx, :], outp[ic][:, 0:F], rcp[:]
                    )
                    i0 = tidx * P
                    nc.gpsimd.dma_start(
                        out=out_ext[i0 : i0 + P, :], in_=o_buf[:, tidx, :]
                    )

    return nc


_DMA_TYPES = (
    "InstDMACopy",
    "InstDmaTransposeAnt",
    "InstDMA",
    "InstDMAGatherAnt",
    "InstDMAScatterAddAnt",
)
_SKIP_TYPES = (
    "InstUnconditionalBranch",
    "InstCall",
    "InstISA",
    "InstCompareAndBranch",
    "InstIndirectBranch",
    "InstHalt",
)


def _split_dma_waits(nc: bass.Bass) -> bass.Bass:
    """walrus' setupSyncWait accepts a limited number of embedded semaphore
    waits per instruction (1 for DMA, 2 for regular TPB instructions);
    hoist any extra waits into NoOps on the issuing engine (engine FIFO
    order makes them complete before the instruction issues)."""
    ctr = 0
    for bb in nc.m.functions[0].blocks:
        new_insts = []
        for ins in bb.instructions:
            t = type(ins).__name__
            si = ins.sync_info
            cap = 1
            if t not in _SKIP_TYPES and si is not None and len(si.on_wait) > cap:
                for w in si.on_wait[:-cap]:
                    ctr += 1
                    new_insts.append(
                        mybir.InstNoOp(
                            name=f"antwait-{ctr}",
                            engine=ins.engine,
                            sync_info=mybir.SyncInfo(on_wait=[w], on_update=[]),
                        )
                    )
                ins.sync_info = mybir.SyncInfo(
                    on_wait=list(si.on_wait[-cap:]), on_update=list(si.on_update)
                )
            new_insts.append(ins)
        bb.instructions = new_insts
    return nc


_CACHED_NC = None


def _get_nc():
    global _CACHED_NC
    if _CACHED_NC is None:
        _CACHED_NC = _split_dma_waits(build_bass())
    return _CACHED_NC


def _bf16(a):
    return np.asarray(a, dtype=np.float32).astype(ml_dtypes.bfloat16)


def kernel(x, Wq, Wk, Wv, _trace=False):
    x = _bf16(x)
    Wq = np.ascontiguousarray(_bf16(Wq))
    Wk = np.ascontiguousarray(_bf16(Wk))
    Wv = np.ascontiguousarray(_bf16(Wv))
    assert x.shape == (N_CORES, S, D)

    nc = _get_nc()
    in_maps = [
        {"x": np.ascontiguousarray(x[b]), "Wq": Wq, "Wk": Wk, "Wv": Wv}
        for b in range(N_CORES)
    ]
    res = run_bass_kernel_spmd(
        nc, in_maps, core_ids=list(range(N_CORES)), trace=_trace
    )
    out = np.stack(
        [res.results[b]["out"].astype(np.float32) for b in range(N_CORES)],
        axis=0,
    )
    if _trace:
        return out, res
    return out
